# revision 29
# baseline (speedup 1.0000x reference)
"""DeepSetObstacles Trainium2 kernel (self-contained).

Reference computation per batch row b (B=65536, M=32 obstacles):
    inp[b,m] = [x[b,2m], x[b,2m+1], vel[b,0], vel[b,1]]          # [4]
    h = relu(relu(inp@W1+b1)@W2+b2)@W3+b3                        # phi: 4->64->64->64
    X[b] = sum_m h[b,m]                                          # [64]
    out[b] = relu(relu(X@R1+rb1)@R2+rb2)@R3+rb3                  # rho: 64->64->64->32

Strategy (pure data parallel over 8 NeuronCores, Bc=8192 rows/core):
  - PE-transpose x to feature-major XT1D[0:64, 8192] (col j = batch row j);
    vel^T lands in rows 64:66 via strided DMA + one cast; row 66 = ones.
  - phi L1: ONE bf16 matmul per obstacle: lhsT [67,64] zero-padded so only
    that obstacle's two x rows + vel rows + ones-row contribute
    (obs@W1o + vel@W1v + b1 in a single accumulation-free matmul).
  - phi L2/L3: 64x64 weight quadrants at 4 tile_positions (concurrent).
  - DeepSets pool: PSUM accumulation in L3; each pool region is fed by
    matmuls of exactly one tile position (cross-position accumulation
    faults the HW).
  - relu+bias fused into single PSUM->SBUF evacuation ops, alternating
    ScalarE activation / VectorE tensor_scalar — the throughput wall.
  - rho on pooled X (2-decker, 512-col chunks), PE-transpose back, DMA out.
"""

import numpy as np

import concourse.bass as bass
import concourse.mybir as mybir
from concourse import bacc
from concourse.tile import TileContext

F32 = mybir.dt.float32
BF16 = mybir.dt.bfloat16
AF = mybir.ActivationFunctionType
ALU = mybir.AluOpType

B, M = 65536, 32
NCORES = 8
Bc = B // NCORES          # 8192 batch rows per core
BLK = 1024
NBLK = Bc // BLK          # 8 blocks over the single-deck column space


class _EvacSplit:
    """Alternate fused relu/bias evacuation ops between ScalarE and VectorE."""

    def __init__(self, nc, act_frac=0.50):
        self.nc = nc
        self.acc = 0.0
        self.act_frac = act_frac

    def _use_act(self):
        self.acc += self.act_frac
        if self.acc >= 1.0:
            self.acc -= 1.0
            return True
        return False

    def relu(self, out_ap, in_ap, bias_ap=None):
        if self._use_act():
            self.nc.scalar.activation(
                out_ap, in_ap, AF.Relu,
                bias=bias_ap if bias_ap is not None else 0.0,
            )
        elif bias_ap is not None:
            self.nc.vector.tensor_scalar(
                out_ap, in_ap, bias_ap, 0.0, ALU.add, ALU.max
            )
        else:
            self.nc.vector.tensor_scalar_max(out_ap, in_ap, 0.0)

    def copy(self, out_ap, in_ap):
        if self._use_act():
            self.nc.scalar.copy(out_ap, in_ap)
        else:
            self.nc.vector.tensor_copy(out_ap, in_ap)

    def linear(self, out_ap, in_ap, bias_ap):
        if self._use_act():
            self.nc.scalar.activation(out_ap, in_ap, AF.Identity, bias=bias_ap)
        else:
            self.nc.vector.tensor_scalar(out_ap, in_ap, bias_ap, None, ALU.add)


def build_nc(repeat=1):
    nc = bacc.Bacc("TRN2", target_bir_lowering=False)

    x_d = nc.declare_dram_parameter("x", [Bc, 64], F32, isOutput=False)
    vel_d = nc.declare_dram_parameter("vel", [Bc, 2], F32, isOutput=False)
    ident_d = nc.declare_dram_parameter("ident", [128, 128], F32, isOutput=False)
    ones_d = nc.declare_dram_parameter("ones_bf", [1, Bc], BF16, isOutput=False)
    w1p_d = nc.declare_dram_parameter("w1p", [128, 2048], BF16, isOutput=False)
    w2bd_d = nc.declare_dram_parameter("w2bd", [128, 128], BF16, isOutput=False)
    w3bd_d = nc.declare_dram_parameter("w3bd", [128, 128], BF16, isOutput=False)
    rw1bd_d = nc.declare_dram_parameter("rw1bd", [128, 128], BF16, isOutput=False)
    rw2bd_d = nc.declare_dram_parameter("rw2bd", [128, 128], BF16, isOutput=False)
    rw3s_d = nc.declare_dram_parameter("rw3s", [128, 128], BF16, isOutput=False)
    idstk_d = nc.declare_dram_parameter("idstk", [128, 128], BF16, isOutput=False)
    bias_d = nc.declare_dram_parameter("biases", [128, 8], F32, isOutput=False)
    out_d = nc.declare_dram_parameter("out", [Bc, 32], F32, isOutput=True)

    with TileContext(nc) as tc:
        with (
            tc.tile_pool(name="const", bufs=1) as const,
            tc.tile_pool(name="xin", bufs=6) as xinp,
            tc.tile_pool(name="big", bufs=1) as big,
            tc.tile_pool(name="h1p", bufs=4) as h1p,
            tc.tile_pool(name="h2p", bufs=4) as h2p,
            tc.tile_pool(name="xap", bufs=2) as xap,
            tc.tile_pool(name="work", bufs=3, space="PSUM") as work,
            tc.tile_pool(name="poolp", bufs=1, space="PSUM") as poolp,
        ):
            ident = const.tile([128, 128], F32)
            nc.sync.dma_start(out=ident[:], in_=ident_d[:])
            w1p = const.tile([128, 2048], BF16)
            nc.sync.dma_start(out=w1p[:], in_=w1p_d[:])
            w2bd = const.tile([128, 128], BF16)
            nc.sync.dma_start(out=w2bd[:], in_=w2bd_d[:])
            w3bd = const.tile([128, 128], BF16)
            nc.sync.dma_start(out=w3bd[:], in_=w3bd_d[:])
            rw1bd = const.tile([128, 128], BF16)
            nc.sync.dma_start(out=rw1bd[:], in_=rw1bd_d[:])
            rw2bd = const.tile([128, 128], BF16)
            nc.sync.dma_start(out=rw2bd[:], in_=rw2bd_d[:])
            rw3s = const.tile([128, 128], BF16)
            nc.sync.dma_start(out=rw3s[:], in_=rw3s_d[:])
            idstk = const.tile([128, 128], BF16)
            nc.sync.dma_start(out=idstk[:], in_=idstk_d[:])
            biases = const.tile([128, 8], F32)
            nc.sync.dma_start(out=biases[:], in_=bias_d[:])
            b2t = biases[:, 1:2]
            rb1e = biases[:, 2:3]
            rb2t = biases[:, 3:4]
            rb3t = biases[:, 4:5]

            # Column order is "chunk-major": col (128u + p) <-> batch (64p + u).
            # phi/pool/rho are column-permutation invariant; the output DMA
            # accounts for the permutation.
            xt1d = big.tile([128, Bc], BF16)     # rows 0:64 x^T, 64:66 vel^T, 66 ones
            vels = big.tile([64, 256], BF16)     # vel chunk-transpose staging
            x2d = big.tile([128, Bc // 2], BF16)  # pooled X, 2-decker chunk=512
            out4d = big.tile([128, 2048], F32)
            outn = big.tile([128, 2048], F32)

            x_rows = x_d.rearrange("(p u) c -> u p c", u=64)   # [64, 128, 64]
            vel_chunks = vel_d.rearrange("(p u) c -> p (u c)", u=64)  # [128, 128]

            for rep in range(repeat):
                ev = _EvacSplit(nc)
                # ---- vel: contiguous load -> 2 strided PE transposes -> scatter
                velc = xinp.tile([128, 128], F32, tag="velc")
                nc.sync.dma_start(out=velc[:], in_=vel_chunks)
                velc_k = velc[:].rearrange("p (u k) -> p u k", k=2)
                tpv = work.tile([128, 1024], F32, tag="work")
                for k in range(2):
                    nc.tensor.transpose(
                        tpv[0:64, 128 * k : 128 * k + 128],
                        velc_k[:, :, k],
                        ident[:],
                    )
                ev.copy(vels[0:64, 0:256], tpv[0:64, 0:256])
                # vels[u, 128k+p] = vel[64p+u, k]  ->  xt1d[64+k, 128u+p]
                for k in range(2):
                    nc.sync.dma_start(
                        out=xt1d[64 + k : 64 + k + 1, :].rearrange(
                            "o (u p) -> o u p", p=128
                        ),
                        in_=vels[0:64, 128 * k : 128 * k + 128],
                    )
                nc.sync.dma_start(out=xt1d[66:67, :], in_=ones_d[:])

                # ---- transpose x -> XT1D rows 0:64 (col 128u+p = batch 64p+u)
                for g in range(8):  # 8 groups x 8 transposes of [128,64]
                    tp = work.tile([128, 1024], F32, tag="work")
                    for i in range(8):
                        u = 8 * g + i
                        xin = xinp.tile([128, 64], F32, tag="xin")
                        nc.sync.dma_start(out=xin[:], in_=x_rows[u])
                        nc.tensor.transpose(
                            tp[0:64, 128 * i : 128 * i + 128], xin[:], ident[:]
                        )
                    ev.copy(
                        xt1d[0:64, 1024 * g : 1024 * g + 1024], tp[0:64, :]
                    )

                # ---- main loop: phi + pool ----
                # l1/h1/l2/h2 tiles [128, 1024]: partition 64a+f = feature f of
                # pair-member a; cols = block columns.
                for blk in range(NBLK):
                    c0 = BLK * blk
                    cols = slice(c0, c0 + BLK)
                    pool = poolp.tile([128, 1024], F32, tag="pool")
                    for pi in range(16):
                        l1 = work.tile([128, 1024], F32, tag="work")
                        for sb_ in range(2):
                            nc.tensor.matmul(
                                l1[:, 512 * sb_ : 512 * sb_ + 512],
                                lhsT=w1p[0:67, 128 * pi : 128 * pi + 128],
                                rhs=xt1d[0:67, c0 + 512 * sb_ :
                                         c0 + 512 * sb_ + 512],
                                start=True,
                                stop=True,
                            )
                        h1 = h1p.tile([128, 1024], BF16, tag="h1")
                        ev.relu(h1[:], l1[:])  # b1 folded into the matmul

                        l2 = work.tile([128, 1024], F32, tag="work")
                        for sb_ in range(2):
                            nc.tensor.matmul(
                                l2[:, 512 * sb_ : 512 * sb_ + 512],
                                lhsT=w2bd[:, :],
                                rhs=h1[:, 512 * sb_ : 512 * sb_ + 512],
                                start=True, stop=True,
                            )
                        h2 = h2p.tile([128, 1024], BF16, tag="h2")
                        ev.relu(h2[:], l2[:], b2t)

                        for sb_ in range(2):
                            nc.tensor.matmul(
                                pool[:, 512 * sb_ : 512 * sb_ + 512],
                                lhsT=w3bd[:, :],
                                rhs=h2[:, 512 * sb_ : 512 * sb_ + 512],
                                start=(pi == 0), stop=(pi == 15),
                                skip_group_check=True,
                            )
                    # fold pool top+bottom via identity-stack matmul; park the
                    # block's X in the xfold psum tile (pair of blocks shares it)
                    pools = h1p.tile([128, 1024], BF16, tag="h1")
                    ev.copy(pools[:], pool[:])
                    if blk % 2 == 0:
                        xfold = work.tile([128, 1024], F32, tag="work")
                    b = blk % 2
                    for sb_ in range(2):
                        nc.tensor.matmul(
                            xfold[64 * b : 64 * b + 64,
                                  512 * sb_ : 512 * sb_ + 512],
                            lhsT=idstk[:, 64 * b : 64 * b + 64],
                            rhs=pools[:, 512 * sb_ : 512 * sb_ + 512],
                            start=True,
                            stop=True,
                            tile_position=(0, 64 * b),
                        )
                    if blk % 2 == 1:
                        e = blk // 2
                        ev.copy(x2d[:, 1024 * e : 1024 * e + 1024], xfold[:])

                # ---- rho (block-diagonal over the two X2D decks) ----
                for rblk in range(4):
                    rc = slice(1024 * rblk, 1024 * rblk + 1024)
                    r1 = work.tile([128, 1024], F32, tag="work")
                    for sb_ in range(2):
                        nc.tensor.matmul(
                            r1[:, 512 * sb_ : 512 * sb_ + 512],
                            lhsT=rw1bd[:, :],
                            rhs=x2d[:, 1024 * rblk + 512 * sb_ :
                                    1024 * rblk + 512 * sb_ + 512],
                            start=True, stop=True,
                        )
                    rh1t = h1p.tile([128, 1024], BF16, tag="h1")
                    ev.relu(rh1t[:], r1[:], rb1e)

                    r2 = work.tile([128, 1024], F32, tag="work")
                    for sb_ in range(2):
                        nc.tensor.matmul(
                            r2[:, 512 * sb_ : 512 * sb_ + 512],
                            lhsT=rw2bd[:, :],
                            rhs=rh1t[:, 512 * sb_ : 512 * sb_ + 512],
                            start=True, stop=True,
                        )
                    rh2t = h2p.tile([128, 1024], BF16, tag="h2")
                    ev.relu(rh2t[:], r2[:], rb2t)

                    if rblk % 2 == 0:
                        r3 = work.tile([128, 1024], F32, tag="work")
                    b = rblk % 2
                    for sb_ in range(2):
                        nc.tensor.matmul(
                            r3[64 * b : 64 * b + 64,
                               512 * sb_ : 512 * sb_ + 512],
                            lhsT=rw3s[:, 64 * b : 64 * b + 64],
                            rhs=rh2t[:, 512 * sb_ : 512 * sb_ + 512],
                            start=True,
                            stop=True,
                            tile_position=(0, 64 * b),
                        )
                    if rblk % 2 == 1:
                        g = rblk // 2
                        ev.linear(
                            out4d[:, 1024 * g : 1024 * g + 1024],
                            r3[:, :],
                            rb3t,
                        )

                # ---- transpose back to row-major and store ----
                for g in range(2):
                    tpo = work.tile([128, 1024], F32, tag="work")
                    for i in range(8):
                        w = 8 * g + i
                        nc.tensor.transpose(
                            tpo[:, 128 * i : 128 * i + 128],
                            out4d[:, 128 * w : 128 * w + 128],
                            ident[:],
                        )
                    ev.copy(outn[:, 1024 * g : 1024 * g + 1024], tpo[:])

                if rep == repeat - 1:
                    # OUT4D partition 32c''+f (c''=2b+a), col 1024g+128(w%8)+uu
                    # batch sigma = 64*uu + K0, K0 = 32g + 16b + 8a + (w%8)
                    out_rows = out_d.rearrange("(u k) c -> k u c", k=64)
                    for w in range(16):
                        for cpp in range(4):
                            b_, a_ = cpp // 2, cpp % 2
                            k0 = 32 * (w // 8) + 16 * b_ + 8 * a_ + (w % 8)
                            nc.sync.dma_start(
                                out=out_rows[k0],
                                in_=outn[:, 128 * w + 32 * cpp :
                                         128 * w + 32 * cpp + 32],
                            )
    nc.compile()
    return nc


def _pack_weights(phi_W1, phi_b1, phi_W2, phi_b2, phi_W3, phi_b3,
                  rho_W1, rho_b1, rho_W2, rho_b2, rho_W3, rho_b3):
    import ml_dtypes

    p = np.arange(128)
    # w1p: pair pi=(2pi, 2pi+1): cols 128pi+64a = weights for obstacle 2pi+a:
    #   rows 2m:2m+2 = W1o, rows 64:66 = W1v, row 66 = b1, else 0
    w1p = np.zeros((128, 2048), np.float32)
    for pi in range(16):
        for a in range(2):
            m = 2 * pi + a
            c0 = 128 * pi + 64 * a
            w1p[2 * m, c0 : c0 + 64] = phi_W1[0, :]
            w1p[2 * m + 1, c0 : c0 + 64] = phi_W1[1, :]
            w1p[64, c0 : c0 + 64] = phi_W1[2, :]
            w1p[65, c0 : c0 + 64] = phi_W1[3, :]
            w1p[66, c0 : c0 + 64] = phi_b1

    def bdiag(wmat):
        out = np.zeros((128, 128), np.float32)
        out[0:64, 0:64] = wmat
        out[64:128, 64:128] = wmat
        return out

    w2bd, w3bd = bdiag(phi_W2), bdiag(phi_W3)
    rw1bd, rw2bd = bdiag(rho_W1), bdiag(rho_W2)
    # rw3s[64a+i, 64b+32a+j] = rho_W3[i, j]  (a deck, b rblk slot)
    rw3s = np.zeros((128, 128), np.float32)
    for a in range(2):
        for b_ in range(2):
            rw3s[64 * a : 64 * a + 64,
                 64 * b_ + 32 * a : 64 * b_ + 32 * a + 32] = rho_W3
    # idstk: vstack(I64, I64) in both col halves
    idstk = np.zeros((128, 128), np.float32)
    for a in range(2):
        for b_ in range(2):
            idstk[64 * a : 64 * a + 64, 64 * b_ : 64 * b_ + 64] = np.eye(64)

    rb1e = rho_b1 + 32.0 * (phi_b3 @ rho_W1)
    biases = np.zeros((128, 8), np.float32)
    biases[:, 0] = phi_b1[p % 64]
    biases[:, 1] = phi_b2[p % 64]
    biases[:, 2] = rb1e[p % 64]
    biases[:, 3] = rho_b2[p % 64]
    biases[:, 4] = rho_b3[p % 32]

    bf = lambda a: np.ascontiguousarray(a, dtype=ml_dtypes.bfloat16)
    return dict(
        ident=np.eye(128, dtype=np.float32),
        ones_bf=bf(np.ones((1, Bc), np.float32)),
        w1p=bf(w1p), w2bd=bf(w2bd), w3bd=bf(w3bd),
        rw1bd=bf(rw1bd), rw2bd=bf(rw2bd), rw3s=bf(rw3s), idstk=bf(idstk),
        biases=biases,
    )


_NC_CACHE = {}


def _get_nc(repeat=1):
    if repeat not in _NC_CACHE:
        _NC_CACHE[repeat] = build_nc(repeat)
    return _NC_CACHE[repeat]


def make_in_maps(x, vel, weights_packed):
    in_maps = []
    for c in range(NCORES):
        m = dict(weights_packed)
        m["x"] = np.ascontiguousarray(x[c * Bc : (c + 1) * Bc], np.float32)
        m["vel"] = np.ascontiguousarray(vel[c * Bc : (c + 1) * Bc], np.float32)
        in_maps.append(m)
    return in_maps


def kernel(x, vel,
           phi_W1, phi_b1, phi_W2, phi_b2, phi_W3, phi_b3,
           rho_W1, rho_b1, rho_W2, rho_b2, rho_W3, rho_b3):
    from concourse.bass_utils import run_bass_kernel_spmd

    wp = _pack_weights(np.asarray(phi_W1), np.asarray(phi_b1),
                       np.asarray(phi_W2), np.asarray(phi_b2),
                       np.asarray(phi_W3), np.asarray(phi_b3),
                       np.asarray(rho_W1), np.asarray(rho_b1),
                       np.asarray(rho_W2), np.asarray(rho_b2),
                       np.asarray(rho_W3), np.asarray(rho_b3))
    in_maps = make_in_maps(np.asarray(x), np.asarray(vel), wp)
    nc = _get_nc(repeat=1)
    res = run_bass_kernel_spmd(nc, in_maps, list(range(NCORES)))
    return np.concatenate([res.results[c]["out"] for c in range(NCORES)], axis=0)
